# revision 1
# baseline (speedup 1.0000x reference)
"""Multi-head attention (B=2, N=2048, DIM=1024, H=16) on 8 Trainium2 NeuronCores.

Sharding: tensor-parallel by head within two quads (cores 0-3 -> batch 0,
cores 4-7 -> batch 1; quad rank r owns heads 4r..4r+3). Each core computes
Q/K/V projections for its 4 heads, masked-softmax attention, then an 8-core
AllToAll re-shards the per-head attention output x^T from head-split to
sequence-split; each core runs the output projection (+bias) for a disjoint
512-token slice and returns that output shard. The host only shards inputs
(transpose + bf16 cast) and concatenates the 8 output shards.

Numerics: matmuls in bf16 with fp32 PSUM accumulation; softmax computed as
exp(SCALE*S)*mask / sum(exp(SCALE*S)*mask) without max-subtraction (scores
are ~N(0,1); exp never overflows fp32). Denominators come from an extra
ones-column appended to V in the attn@V matmul (column 64 for the even head
of a pair, 96 for the odd head, so the sums land on 32-aligned PSUM
partitions and every vector-engine op keeps matching partition bases).
Measured end-to-end L2 relative error vs the f32 reference: ~6e-3.
"""

import numpy as np
import ml_dtypes

import concourse.bass as bass
import concourse.mybir as mybir
import concourse.tile as tile
from concourse.masks import make_identity

F32 = mybir.dt.float32
BF16 = mybir.dt.bfloat16
BF16_NP = ml_dtypes.bfloat16

B, DIM, H = 2, 1024, 16
N_FULL = 2048
HD = DIM // H          # 64
SCALE = HD ** -0.5     # 0.125
NCORES = 8
H_LOC = H // 4         # 4 heads per core
COLS = H_LOC * HD      # 256 local channels
KT_D = DIM // 128      # 8 contraction tiles over DIM
GROUPS = [list(range(NCORES))]


# ---------------------------------------------------------------------------
# Workaround: this walrus build rejects >2 sync waits on one instruction
# ("Too many sync wait commands" in setupSyncWait). The TileContext final
# drain aggregates one wait per logical processor; split it into a chain of
# single-wait drains.
# ---------------------------------------------------------------------------
def _patch_tile_drain():
    from bass_rust import ScopedClock

    if getattr(tile.TileContext, "_drain_patched", False):
        return

    def _drain_and_barrier(self, tick_clock, wait_clock):
        nc = self.nc
        drain_inst = nc.sync.drain()
        wait_clock.add_sem_waits(
            drain_inst.ins, ScopedClock({None: tick_clock.global_clock})
        )
        si = drain_inst.ins.sync_info
        if si is not None and len(si.on_wait) > 1:
            waits = list(si.on_wait)
            drain_inst.ins.sync_info = mybir.SyncInfo(
                on_wait=waits[:1], on_update=list(si.on_update)
            )
            for w in waits[1:]:
                d = nc.sync.drain()
                dsi = d.ins.sync_info
                upd = list(dsi.on_update) if dsi is not None else []
                d.ins.sync_info = mybir.SyncInfo(on_wait=[w], on_update=upd)

        nc.all_engine_barrier()
        assert self.sems is not None
        popped = nc._tile_sem_poison_stack.pop()
        assert popped is self._sem_poison
        nc.clear_and_free_semaphores(list(self.sems.allocated().values()))
        nc.all_engine_barrier()

    tile.TileContext._drain_and_barrier = _drain_and_barrier
    tile.TileContext._drain_patched = True


def _split_sync_waits(nc, maxw=1):
    """Walrus in this build rejects instructions carrying more than a couple
    of semaphore waits. Move excess waits onto injected same-engine NoOps
    immediately before the instruction (identical semantics: the engine
    blocks at the nop instead of at the instruction itself)."""
    n_split = 0
    for f in nc.m.functions:
        for bb in f.blocks:
            new_insts = []
            for ins in bb.instructions:
                si = ins.sync_info
                if si is not None and len(si.on_wait) > maxw:
                    waits = list(si.on_wait)
                    for i, w in enumerate(waits[maxw:]):
                        nop = mybir.InstNoOp(
                            name=f"{ins.name}-w{i}", ins=[], outs=[]
                        )
                        nop.engine = ins.engine
                        nop.sync_info = mybir.SyncInfo(
                            on_wait=[w], on_update=[]
                        )
                        new_insts.append(nop)
                    ins.sync_info = mybir.SyncInfo(
                        on_wait=waits[:maxw], on_update=list(si.on_update)
                    )
                    n_split += 1
                new_insts.append(ins)
            bb.instructions = new_insts
    return n_split


def build_nc(N=N_FULL, split_waits=True):
    """Build the per-core Bass program (same SPMD program for all 8 cores).

    N is parameterizable (multiple of 512) so a scaled-down variant can be
    validated in the simulator; the graded configuration is N=2048.
    """
    _patch_tile_drain()
    assert N % 512 == 0
    NSLICE = N // 4            # output rows per core
    MT = N // 128              # m-tiles over keys
    HS = min(N, 1024)          # attention n-chunk size
    NH = N // HS               # number of n-chunks in phase 2
    NT = NSLICE // 128         # output row tiles
    NCH = N // 512             # 512-col chunks of N
    HC = HS // 512             # 512-col chunks of one n-chunk

    def nsl_of(nh):
        return slice(HS * nh, HS * (nh + 1))

    nc = bass.Bass(trn_type="TRN2", num_devices=NCORES)

    xqT_e = nc.declare_dram_parameter("xqT", [DIM, N], BF16, isOutput=False)
    xkT_e = nc.declare_dram_parameter("xkT", [DIM, N], BF16, isOutput=False)
    xvT_e = nc.declare_dram_parameter("xvT", [DIM, N], BF16, isOutput=False)
    wq_e = nc.declare_dram_parameter("wq", [DIM, COLS], BF16, isOutput=False)
    wk_e = nc.declare_dram_parameter("wk", [DIM, COLS], BF16, isOutput=False)
    wv_e = nc.declare_dram_parameter("wv", [DIM, COLS], BF16, isOutput=False)
    wpp_e = nc.declare_dram_parameter("wp_pad", [2 * DIM, DIM], BF16, isOutput=False)
    maskT_e = nc.declare_dram_parameter("maskT", [N, N], BF16, isOutput=False)
    bpr_e = nc.declare_dram_parameter("bp_rep", [128, DIM], F32, isOutput=False)
    out_e = nc.declare_dram_parameter("out", [NSLICE, DIM], F32, isOutput=True)

    a2a_in = nc.dram_tensor("a2a_in", [NCORES * COLS, NSLICE], BF16)
    a2a_out = nc.dram_tensor("a2a_out", [NCORES * COLS, NSLICE], BF16)

    with tile.TileContext(nc) as tc:
        with (
            tc.tile_pool(name="cpool", bufs=1) as cpool,
            tc.tile_pool(name="xstream", bufs=3) as xpool,
            tc.tile_pool(name="pupool", bufs=3) as pupool,
            tc.tile_pool(name="yupool", bufs=4) as yupool,
            tc.tile_pool(name="p3pool", bufs=2) as p3pool,
            tc.tile_pool(name="opool", bufs=2) as opool,
            tc.tile_pool(name="ps", bufs=1, space="PSUM") as ps,
        ):
            # PSUM: four 2-bank (4KB/partition) tag slots shared by all
            # phases; static pool allocation = 8 banks.
            PST = [f"PS{i}" for i in range(4)]

            # ---- long-lived SBUF tensors -------------------------------
            qt_sb = [cpool.tile([128, N], BF16, tag=f"qt{i}", name=f"qt{i}")
                     for i in range(2)]
            kt_sb = [cpool.tile([128, N], BF16, tag=f"kt{i}", name=f"kt{i}")
                     for i in range(2)]
            # V per m-tile: [m, head, 65]; cols 0..63 = V_head, col 64 = ones
            vt_sb = [cpool.tile([128, H_LOC, 65], BF16, tag=f"vt{t}",
                                name=f"vt{t}")
                     for t in range(MT)]
            # per-local-head attention output x^T, partitions 0..63
            xt_sb = [cpool.tile([64, N], BF16, tag=f"xth{g}", name=f"xth{g}")
                     for g in range(H_LOC)]
            ones_sb = cpool.tile([128, 64], F32, tag="ones", name="ones")
            ident_sb = cpool.tile([128, 128], BF16, tag="ident", name="ident")
            r_sbs = [cpool.tile([65, HS], F32, tag=f"rsum{h}", name=f"rsum{h}")
                     for h in range(2)]
            rr_sb = [cpool.tile([64, HS], BF16, tag=f"rr{h}", name=f"rr{h}")
                     for h in range(2)]
            mask_sb = cpool.tile([128, MT, N], BF16, tag="mask", name="mask")
            bpr_sb = cpool.tile([128, DIM], F32, tag="bpr", name="bpr")
            wq_sb = cpool.tile([128, KT_D, COLS], BF16, tag="wq", name="wq")
            wk_sb = cpool.tile([128, KT_D, COLS], BF16, tag="wk", name="wk")
            wv_sb = cpool.tile([128, KT_D, COLS], BF16, tag="wv", name="wv")
            xv_sb = cpool.tile([128, KT_D, N], BF16, tag="xv", name="xv")

            # weights + constants
            wq_v = wq_e[:].rearrange("(kt p) c -> p kt c", p=128)
            wk_v = wk_e[:].rearrange("(kt p) c -> p kt c", p=128)
            wv_v = wv_e[:].rearrange("(kt p) c -> p kt c", p=128)
            nc.sync.dma_start(wq_sb[:], wq_v)
            nc.sync.dma_start(wk_sb[:], wk_v)
            nc.sync.dma_start(wv_sb[:], wv_v)
            nc.sync.dma_start(bpr_sb[:], bpr_e[:])
            nc.gpsimd.memset(ones_sb[:], 0.0)
            nc.gpsimd.memset(ones_sb[64:65, :], 1.0)
            make_identity(nc, ident_sb[:])
            for t in range(MT):
                nc.gpsimd.memset(vt_sb[t][:, :, 64:65], 1.0)

            # ---- phase 1: projections ----------------------------------
            # Q^T and K^T: [COLS, N] as two 128-row blocks; kt-outer with
            # one live [128, <=1024] psum accumulator per (block, n-half).
            NH2 = max(1, N // 1024)
            W2 = min(N, 1024)
            for w_sb, x_e, dst in (
                (wq_sb, xqT_e, qt_sb),
                (wk_sb, xkT_e, kt_sb),
            ):
                psums = [ps.tile([128, W2], F32, tag=PST[cb * NH2 + n2],
                                 name="p1qk")
                         for cb in range(2) for n2 in range(NH2)]
                for kt in range(KT_D):
                    xt_t = xpool.tile([128, N], BF16, tag="xs", name="xs")
                    nc.sync.dma_start(xt_t[:], x_e[128 * kt:128 * (kt + 1), :])
                    for cb in range(2):
                        for nch in range(NCH):
                            n2, ch = divmod(nch, W2 // 512)
                            nc.tensor.matmul(
                                psums[cb * NH2 + n2][:, 512 * ch:512 * (ch + 1)],
                                w_sb[:, kt, 128 * cb:128 * (cb + 1)],
                                xt_t[:, 512 * nch:512 * (nch + 1)],
                                start=(kt == 0), stop=(kt == KT_D - 1),
                            )
                for cb in range(2):
                    for n2 in range(NH2):
                        nc.scalar.copy(
                            dst[cb][:, W2 * n2:W2 * (n2 + 1)],
                            psums[cb * NH2 + n2][:],
                        )

            # V in natural layout: out[m-tile, 4*HD] = xvT_kt^T @ wv_kt
            xv_v = xvT_e[:].rearrange("(kt p) n -> p kt n", p=128)
            nc.sync.dma_start(xv_sb[:], xv_v)
            for t in range(MT):
                nc.sync.dma_start(
                    mask_sb[:, t, :], maskT_e[128 * t:128 * (t + 1), :]
                )
            for t in range(MT):
                vps = ps.tile([128, COLS], F32, tag=PST[t % 2], name="p1v")
                for kt in range(KT_D):
                    nc.tensor.matmul(
                        vps[:],
                        xv_sb[:, kt, 128 * t:128 * (t + 1)],
                        wv_sb[:, kt, :],
                        start=(kt == 0), stop=(kt == KT_D - 1),
                    )
                nc.scalar.copy(
                    vt_sb[t][:, :, 0:HD],
                    vps[:].rearrange("p (h d) -> p h d", h=H_LOC),
                )

            # ---- phase 2: attention ------------------------------------
            # Head pairs; scores + additive mask (identity matmul) in PSUM;
            # exp on ScalarE; attn@[V|ones] accumulation; the VO accumulator
            # is evicted to SBUF immediately (one ScalarE copy) so the
            # normalization chain runs in the background off the vector
            # engine while the next pass's matmuls own the PSUM.
            for nh in range(NH):
                nsl = nsl_of(nh)
                for hp in range(2):
                    vo = [ps.tile([65, HS], F32, tag=PST[2 + h], name="vo")
                          for h in range(2)]
                    for t in range(MT):
                        s_ps = [ps.tile([128, HS], F32, tag=PST[h], name="s")
                                for h in range(2)]
                        # score matmul pairs adjacent so the K=64 row-group
                        # concurrency engages (measured 1.75x vs serial)
                        for ch in range(HC):
                            csl = slice(512 * ch, 512 * (ch + 1))
                            gsl = slice(HS * nh + 512 * ch,
                                        HS * nh + 512 * (ch + 1))
                            for h in range(2):
                                nc.tensor.matmul(
                                    s_ps[h][:, csl],
                                    kt_sb[hp][64 * h:64 * (h + 1),
                                              128 * t:128 * (t + 1)],
                                    qt_sb[hp][64 * h:64 * (h + 1), gsl],
                                    start=True, stop=False,
                                    tile_position=(64 * h, 0),
                                )
                        for ch in range(HC):
                            csl = slice(512 * ch, 512 * (ch + 1))
                            gsl = slice(HS * nh + 512 * ch,
                                        HS * nh + 512 * (ch + 1))
                            for h in range(2):
                                nc.tensor.matmul(
                                    s_ps[h][:, csl],
                                    ident_sb[:],
                                    mask_sb[:, t, gsl],
                                    start=False, stop=True,
                                )
                        for h in range(2):
                            pu = pupool.tile([128, HS], BF16, tag="pu",
                                             name="pu")
                            nc.scalar.activation(
                                pu[:], s_ps[h][:],
                                mybir.ActivationFunctionType.Exp,
                                scale=float(SCALE),
                            )
                            for ch in range(HC):
                                csl = slice(512 * ch, 512 * (ch + 1))
                                nc.tensor.matmul(
                                    vo[h][:, csl],
                                    vt_sb[t][:, 2 * hp + h, :],
                                    pu[:, csl],
                                    start=(t == 0), stop=(t == MT - 1),
                                )
                    # evict unnormalized y fast, then normalize off-PSUM
                    for h in range(2):
                        yu = yupool.tile([65, HS], BF16, tag="yu", name="yu")
                        with nc.allow_low_precision(reason="softmax y bf16"):
                            nc.scalar.copy(yu[:], vo[h][:])
                        nc.vector.reciprocal(
                            r_sbs[h][64:65, :], yu[64:65, :]
                        )
                        rr_ps = ps.tile([64, HS], F32, tag=PST[h], name="rr")
                        for ch in range(HC):
                            csl = slice(512 * ch, 512 * (ch + 1))
                            nc.tensor.matmul(
                                rr_ps[:, csl],
                                ones_sb[64:65, :],
                                r_sbs[h][64:65, csl],
                                start=True, stop=True,
                            )
                        with nc.allow_low_precision(reason="softmax norm bf16"):
                            nc.vector.tensor_copy(rr_sb[h][:], rr_ps[:])
                        nc.vector.tensor_mul(
                            xt_sb[2 * hp + h][:, nsl],
                            yu[0:64, :],
                            rr_sb[h][:],
                        )
            # ---- phase 3: AllToAll + output projection -----------------
            a2a_in_v = a2a_in[:].rearrange("(j g p) n -> j g p n",
                                           j=NCORES, g=H_LOC)
            for jj in range(NCORES):
                sl = slice(NSLICE * (jj % 4), NSLICE * (jj % 4 + 1))
                for g in range(H_LOC):
                    nc.sync.dma_start(a2a_in_v[jj, g], xt_sb[g][:, sl])
            nc.gpsimd.collective_compute(
                "AllToAll",
                mybir.AluOpType.bypass,
                replica_groups=GROUPS,
                ins=[a2a_in[:]],
                outs=[a2a_out[:]],
            )
            pj = [ps.tile([128, DIM], F32, tag=PST[nt], name=f"pj{nt}")
                  for nt in range(NT)]
            a2a_out_v = a2a_out[:].rearrange("(ct p) n -> p ct n", p=128)
            wpp_v = wpp_e[:].rearrange("(ct p) c -> p ct c", p=128)
            for ct in range(2 * KT_D):
                aa_t = p3pool.tile([128, NSLICE], BF16, tag="aa", name="aa")
                nc.sync.dma_start(aa_t[:], a2a_out_v[:, ct, :])
                wp_t = p3pool.tile([128, DIM], BF16, tag="wp", name="wp")
                nc.sync.dma_start(wp_t[:], wpp_v[:, ct, :])
                for nt in range(NT):
                    for ch in range(2):
                        nc.tensor.matmul(
                            pj[nt][:, 512 * ch:512 * (ch + 1)],
                            aa_t[:, 128 * nt:128 * (nt + 1)],
                            wp_t[:, 512 * ch:512 * (ch + 1)],
                            start=(ct == 0), stop=(ct == 2 * KT_D - 1),
                        )
            for nt in range(NT):
                o_t = opool.tile([128, DIM], F32, tag="ot", name="ot")
                for ch in range(2):
                    csl = slice(512 * ch, 512 * (ch + 1))
                    nc.vector.tensor_add(
                        o_t[:, csl], pj[nt][:, csl], bpr_sb[:, csl]
                    )
                nc.sync.dma_start(out_e[128 * nt:128 * (nt + 1), :], o_t[:])

    if split_waits:
        _split_sync_waits(nc)
    return nc


def make_in_maps(q, k, v, mask, Wq, Wk, Wv, Wp, bp, N=N_FULL):
    """Shard + pre-transpose + bf16-cast the full inputs for the 8 cores."""
    bf = lambda a: np.ascontiguousarray(a).astype(BF16_NP)
    bp_rep = np.ascontiguousarray(
        np.broadcast_to(bp.astype(np.float32), (128, DIM))
    )
    in_maps = []
    for c in range(NCORES):
        b, r = divmod(c, 4)
        cs = slice(COLS * r, COLS * (r + 1))
        wp_pad = np.zeros((2 * DIM, DIM), np.float32)
        wp_pad[DIM * b:DIM * (b + 1)] = Wp
        in_maps.append({
            "xqT": bf(q[b].T),
            "xkT": bf(k[b].T),
            "xvT": bf(v[b].T),
            "wq": bf(Wq[:, cs]),
            "wk": bf(Wk[:, cs]),
            "wv": bf(Wv[:, cs]),
            "wp_pad": bf(wp_pad),
            "maskT": bf((mask[b, 0].T.astype(np.float32) - 1.0) * 1e6),
            "bp_rep": bp_rep,
        })
    return in_maps


def assemble_out(results, N=N_FULL):
    NSLICE = N // 4
    out = np.empty((B, N, DIM), np.float32)
    for c in range(NCORES):
        b, r = divmod(c, 4)
        out[b, NSLICE * r:NSLICE * (r + 1), :] = results[c]["out"]
    return out


_NC_CACHE = {}


def _get_nc():
    if "nc" not in _NC_CACHE:
        _NC_CACHE["nc"] = build_nc()
    return _NC_CACHE["nc"]


def kernel(q, k, v, mask, Wq, Wk, Wv, Wp, bp):
    from concourse.bass_utils import run_bass_kernel_spmd

    q, k, v = (np.asarray(a, np.float32) for a in (q, k, v))
    mask = np.asarray(mask)
    Wq, Wk, Wv, Wp, bp = (
        np.asarray(a, np.float32) for a in (Wq, Wk, Wv, Wp, bp)
    )
    nc = _get_nc()
    in_maps = make_in_maps(q, k, v, mask, Wq, Wk, Wv, Wp, bp)
    res = run_bass_kernel_spmd(nc, in_maps, core_ids=list(range(NCORES)))
    return assemble_out(res.results)



# revision 19
# speedup vs baseline: 1.1536x; 1.1536x over previous
"""Multi-head attention (B=2, N=2048, DIM=1024, H=16) on 8 Trainium2 NeuronCores.

Sharding: tensor-parallel by head within two quads (cores 0-3 -> batch 0,
cores 4-7 -> batch 1; quad rank r owns heads 4r..4r+3). Each core computes
Q/K/V projections for its 4 heads, masked-softmax attention, then a
quad-local AllToAll re-shards the per-head attention output x^T from
head-split to sequence-split; each core runs the output projection (+bias)
for a disjoint 512-token slice and returns that output shard. The host only
shards inputs (transpose + bf16 cast) and concatenates the 8 output shards.

The head-pair loop (hp) is outermost so the AllToAll can be issued in two
chunks: chunk A (heads 0-1 of each rank) goes out while the PE computes
heads 2-3, hiding most of the collective latency. The output projection
accumulates chunk-A channels first, then chunk-B channels, with the weight
rows pre-reordered host-side to match the interleaved channel order.

Numerics: matmuls in bf16 with fp32 PSUM accumulation; softmax computed as
exp(SCALE*S)*mask / sum(exp(SCALE*S)*mask) without max-subtraction (scores
are ~N(0,1); exp never overflows fp32). Denominators come from an extra
ones-column appended to V in the attn@V matmul (row 64 of the 65-partition
VO accumulator). Normalization runs off-PSUM: reciprocal_approx_fast on the
denominator row, a bf16 ones-broadcast matmul, and one DVE multiply reading
the VO PSUM directly (no intermediate eviction).
Measured end-to-end L2 relative error vs the f32 reference: ~6e-3.
"""

import numpy as np
import ml_dtypes

import concourse.bass as bass
import concourse.mybir as mybir
import concourse.tile as tile
from concourse.masks import make_identity

F32 = mybir.dt.float32
BF16 = mybir.dt.bfloat16
BF16_NP = ml_dtypes.bfloat16

B, DIM, H = 2, 1024, 16
N_FULL = 2048
HD = DIM // H          # 64
SCALE = HD ** -0.5     # 0.125
NCORES = 8
H_LOC = H // 4         # 4 heads per core
COLS = H_LOC * HD      # 256 local channels
KT_D = DIM // 128      # 8 contraction tiles over DIM
GROUPS = [list(range(NCORES))]   # AllToAll needs mesh routing (>4 ranks)
QUAD = 4


# ---------------------------------------------------------------------------
# Workaround: this walrus build rejects >2 sync waits on one instruction
# ("Too many sync wait commands" in setupSyncWait). The TileContext final
# drain aggregates one wait per logical processor; split it into a chain of
# single-wait drains.
# ---------------------------------------------------------------------------
def _patch_tile_drain():
    from bass_rust import ScopedClock

    if getattr(tile.TileContext, "_drain_patched", False):
        return

    def _drain_and_barrier(self, tick_clock, wait_clock):
        nc = self.nc
        drain_inst = nc.sync.drain()
        wait_clock.add_sem_waits(
            drain_inst.ins, ScopedClock({None: tick_clock.global_clock})
        )
        si = drain_inst.ins.sync_info
        if si is not None and len(si.on_wait) > 1:
            waits = list(si.on_wait)
            drain_inst.ins.sync_info = mybir.SyncInfo(
                on_wait=waits[:1], on_update=list(si.on_update)
            )
            for w in waits[1:]:
                d = nc.sync.drain()
                dsi = d.ins.sync_info
                upd = list(dsi.on_update) if dsi is not None else []
                d.ins.sync_info = mybir.SyncInfo(on_wait=[w], on_update=upd)

        nc.all_engine_barrier()
        assert self.sems is not None
        popped = nc._tile_sem_poison_stack.pop()
        assert popped is self._sem_poison
        nc.clear_and_free_semaphores(list(self.sems.allocated().values()))
        nc.all_engine_barrier()

    tile.TileContext._drain_and_barrier = _drain_and_barrier
    tile.TileContext._drain_patched = True


def _split_sync_waits(nc, maxw=1):
    """Walrus in this build rejects instructions carrying more than a couple
    of semaphore waits. Move excess waits onto injected same-engine NoOps
    immediately before the instruction (identical semantics: the engine
    blocks at the nop instead of at the instruction itself)."""
    n_split = 0
    for f in nc.m.functions:
        for bb in f.blocks:
            new_insts = []
            for ins in bb.instructions:
                si = ins.sync_info
                if si is not None and len(si.on_wait) > maxw:
                    waits = list(si.on_wait)
                    for i, w in enumerate(waits[maxw:]):
                        nop = mybir.InstNoOp(
                            name=f"{ins.name}-w{i}", ins=[], outs=[]
                        )
                        nop.engine = ins.engine
                        nop.sync_info = mybir.SyncInfo(
                            on_wait=[w], on_update=[]
                        )
                        new_insts.append(nop)
                    ins.sync_info = mybir.SyncInfo(
                        on_wait=waits[:maxw], on_update=list(si.on_update)
                    )
                    n_split += 1
                new_insts.append(ins)
            bb.instructions = new_insts
    return n_split


def build_nc(N=N_FULL, split_waits=True):
    """Build the per-core Bass program (same SPMD program for all 8 cores).

    N is parameterizable (multiple of 512) so a scaled-down variant can be
    validated in the simulator; the graded configuration is N=2048.
    """
    _patch_tile_drain()
    assert N % 512 == 0
    NSLICE = N // 4            # output rows per core
    MT = N // 128              # m-tiles over keys
    HS = min(N, 1024)          # attention n-chunk size
    NH = N // HS               # number of n-chunks in phase 2
    NT = NSLICE // 128         # output row tiles
    NCH = N // 512             # 512-col chunks of N
    HC = HS // 512             # 512-col chunks of one n-chunk

    nc = bass.Bass(trn_type="TRN2", num_devices=NCORES)

    xqT_e = nc.declare_dram_parameter("xqT", [DIM, N], BF16, isOutput=False)
    xkT_e = nc.declare_dram_parameter("xkT", [DIM, N], BF16, isOutput=False)
    xvT_e = nc.declare_dram_parameter("xvT", [DIM, N], BF16, isOutput=False)
    wq_e = nc.declare_dram_parameter("wq", [DIM, COLS], BF16, isOutput=False)
    wk_e = nc.declare_dram_parameter("wk", [DIM, COLS], BF16, isOutput=False)
    wv_e = nc.declare_dram_parameter("wv", [DIM, COLS], BF16, isOutput=False)
    # wp rows host-reordered/zero-padded to the chunked AllToAll channel
    # order: chunk A (heads 0-1 of each source rank) then chunk B (heads
    # 2-3), with other-quad source rows zeroed (their payload is the other
    # batch's data).
    wp_e = nc.declare_dram_parameter("wp", [2 * DIM, DIM], BF16, isOutput=False)
    maskT_e = nc.declare_dram_parameter("maskT", [N, N], BF16, isOutput=False)
    bpr_e = nc.declare_dram_parameter("bp_rep", [128, DIM], F32, isOutput=False)
    out_e = nc.declare_dram_parameter("out", [NSLICE, DIM], F32, isOutput=True)

    # per-chunk AllToAll buffers: 2 heads x 8 ranks, [8 dest * 2 g * 64, NSLICE]
    CH_ROWS = NCORES * 2 * HD    # 1024
    a2a_in = [nc.dram_tensor(f"a2a_in{c}", [CH_ROWS, NSLICE], BF16)
              for c in range(2)]
    a2a_out = [nc.dram_tensor(f"a2a_out{c}", [CH_ROWS, NSLICE], BF16)
               for c in range(2)]
    rscr = [nc.dram_tensor(f"rscr{h}", [1, min(N, 1024)], F32)
            for h in range(2)]

    with tile.TileContext(nc) as tc:
        with (
            tc.tile_pool(name="cpool", bufs=1) as cpool,
            tc.tile_pool(name="xstream", bufs=3) as xpool,
            tc.tile_pool(name="pupool", bufs=3) as pupool,
            tc.tile_pool(name="p3pool", bufs=3) as p3pool,
            tc.tile_pool(name="opool", bufs=2) as opool,
            tc.tile_pool(name="ps", bufs=1, space="PSUM") as ps,
        ):
            # PSUM: four 2-bank (4KB/partition) tag slots shared by all
            # phases; static pool allocation = 8 banks.
            PST = [f"PS{i}" for i in range(4)]

            # ---- long-lived SBUF tensors -------------------------------
            qt_sb = [cpool.tile([128, N], BF16, tag=f"qt{i}", name=f"qt{i}")
                     for i in range(2)]
            kt_sb = [cpool.tile([128, N], BF16, tag=f"kt{i}", name=f"kt{i}")
                     for i in range(2)]
            # V per m-tile: [m, head, 65]; cols 0..63 = V_head, col 64 = ones
            vt_sb = [cpool.tile([128, H_LOC, 65], BF16, tag=f"vt{t}",
                                name=f"vt{t}")
                     for t in range(MT)]
            # per-local-head attention output x^T, partitions 0..63
            xt_sb = [cpool.tile([64, N], BF16, tag=f"xth{g}", name=f"xth{g}")
                     for g in range(H_LOC)]
            ident_sb = cpool.tile([128, 128], BF16, tag="ident", name="ident")
            rf_sb = [cpool.tile([1, HS], F32, tag=f"rf{h}", name=f"rf{h}")
                     for h in range(2)]
            rr_sb = [cpool.tile([64, HS], F32, tag=f"rr{h}", name=f"rr{h}")
                     for h in range(2)]
            mask_sb = cpool.tile([128, MT, N], BF16, tag="mask", name="mask")
            bpr_sb = cpool.tile([128, DIM], F32, tag="bpr", name="bpr")
            wq_sb = cpool.tile([128, KT_D, COLS], BF16, tag="wq", name="wq")
            wk_sb = cpool.tile([128, KT_D, COLS], BF16, tag="wk", name="wk")
            wv_sb = cpool.tile([128, KT_D, COLS], BF16, tag="wv", name="wv")
            xv_sb = cpool.tile([128, KT_D, N], BF16, tag="xv", name="xv")

            # weights + constants
            wq_v = wq_e[:].rearrange("(kt p) c -> p kt c", p=128)
            wk_v = wk_e[:].rearrange("(kt p) c -> p kt c", p=128)
            wv_v = wv_e[:].rearrange("(kt p) c -> p kt c", p=128)
            nc.sync.dma_start(wq_sb[:], wq_v)
            nc.sync.dma_start(wk_sb[:], wk_v)
            nc.sync.dma_start(wv_sb[:], wv_v)
            nc.sync.dma_start(bpr_sb[:], bpr_e[:])
            make_identity(nc, ident_sb[:])
            for t in range(MT):
                nc.gpsimd.memset(vt_sb[t][:, :, 64:65], 1.0)

            # ---- phase 1: projections ----------------------------------
            # Q^T and K^T: [COLS, N] as two 128-row blocks; kt-outer with
            # one live [128, <=1024] psum accumulator per (block, n-half).
            NH2 = max(1, N // 1024)
            W2 = min(N, 1024)
            for w_sb, x_e, dst in (
                (wq_sb, xqT_e, qt_sb),
                (wk_sb, xkT_e, kt_sb),
            ):
                psums = [ps.tile([128, W2], F32, tag=PST[cb * NH2 + n2],
                                 name="p1qk")
                         for cb in range(2) for n2 in range(NH2)]
                for kt in range(KT_D):
                    xt_t = xpool.tile([128, N], BF16, tag="xs", name="xs")
                    nc.sync.dma_start(xt_t[:], x_e[128 * kt:128 * (kt + 1), :])
                    for cb in range(2):
                        for nch in range(NCH):
                            n2, ch = divmod(nch, W2 // 512)
                            nc.tensor.matmul(
                                psums[cb * NH2 + n2][:, 512 * ch:512 * (ch + 1)],
                                w_sb[:, kt, 128 * cb:128 * (cb + 1)],
                                xt_t[:, 512 * nch:512 * (nch + 1)],
                                start=(kt == 0), stop=(kt == KT_D - 1),
                            )
                for cb in range(2):
                    for n2 in range(NH2):
                        nc.scalar.copy(
                            dst[cb][:, W2 * n2:W2 * (n2 + 1)],
                            psums[cb * NH2 + n2][:],
                        )

            # V in natural layout: out[m-tile, 4*HD] = xvT_kt^T @ wv_kt
            xv_v = xvT_e[:].rearrange("(kt p) n -> p kt n", p=128)
            nc.sync.dma_start(xv_sb[:], xv_v)
            for t in range(MT):
                nc.sync.dma_start(
                    mask_sb[:, t, :], maskT_e[128 * t:128 * (t + 1), :]
                )
            for t in range(MT):
                vps = ps.tile([128, COLS], F32, tag=PST[t % 2], name="p1v")
                for kt in range(KT_D):
                    nc.tensor.matmul(
                        vps[:],
                        xv_sb[:, kt, 128 * t:128 * (t + 1)],
                        wv_sb[:, kt, :],
                        start=(kt == 0), stop=(kt == KT_D - 1),
                    )
                nc.scalar.copy(
                    vt_sb[t][:, :, 0:HD],
                    vps[:].rearrange("p (h d) -> p h d", h=H_LOC),
                )

            # ---- phase 2: attention (hp outer for chunked collective) --
            # Scores + additive mask (identity matmul) in PSUM; exp on
            # ScalarE; attn@[V|ones] accumulation; normalization reads the
            # VO accumulator straight from PSUM (reciprocal_approx_fast ->
            # bf16 ones-broadcast matmul -> one DVE multiply into xt).
            for hp in range(2):
                for nh in range(NH):
                    nsl = slice(HS * nh, HS * (nh + 1))
                    vo = [ps.tile([65, HS], F32, tag=PST[2 + h], name="vo")
                          for h in range(2)]
                    for t in range(MT):
                        s_ps = [ps.tile([128, HS], F32, tag=PST[h], name="s")
                                for h in range(2)]
                        # score matmul pairs adjacent so the K=64 row-group
                        # concurrency engages (measured 1.75x vs serial)
                        for ch in range(HC):
                            csl = slice(512 * ch, 512 * (ch + 1))
                            gsl = slice(HS * nh + 512 * ch,
                                        HS * nh + 512 * (ch + 1))
                            for h in range(2):
                                nc.tensor.matmul(
                                    s_ps[h][:, csl],
                                    kt_sb[hp][64 * h:64 * (h + 1),
                                              128 * t:128 * (t + 1)],
                                    qt_sb[hp][64 * h:64 * (h + 1), gsl],
                                    start=True, stop=False,
                                    tile_position=(64 * h, 0),
                                )
                        for ch in range(HC):
                            csl = slice(512 * ch, 512 * (ch + 1))
                            gsl = slice(HS * nh + 512 * ch,
                                        HS * nh + 512 * (ch + 1))
                            for h in range(2):
                                nc.tensor.matmul(
                                    s_ps[h][:, csl],
                                    ident_sb[:],
                                    mask_sb[:, t, gsl],
                                    start=False, stop=True,
                                )
                        for h in range(2):
                            pu = pupool.tile([128, HS], BF16, tag="pu",
                                             name="pu")
                            nc.scalar.activation(
                                pu[:], s_ps[h][:],
                                mybir.ActivationFunctionType.Exp,
                                scale=float(SCALE),
                            )
                            for ch in range(HC):
                                csl = slice(512 * ch, 512 * (ch + 1))
                                nc.tensor.matmul(
                                    vo[h][:, csl],
                                    vt_sb[t][:, 2 * hp + h, :],
                                    pu[:, csl],
                                    start=(t == 0), stop=(t == MT - 1),
                                )
                    # normalize straight from PSUM: r = 1/denominator on DVE,
                    # broadcast r to 64 partitions via a DRAM round-trip DMA
                    # (SBUF APs can't 0-stride; DRAM-side broadcast can),
                    # then one DVE multiply (vo stays in PSUM, no eviction,
                    # and the score PSUM tags are never touched).
                    for h in range(2):
                        nc.vector.reciprocal(rf_sb[h][:], vo[h][64:65, :])
                        nc.sync.dma_start(rscr[h][:], rf_sb[h][:])
                        nc.sync.dma_start(
                            rr_sb[h][:], rscr[h][:].broadcast_to((64, HS))
                        )
                        with nc.allow_low_precision(reason="softmax y bf16"):
                            nc.vector.tensor_mul(
                                xt_sb[2 * hp + h][:, nsl],
                                vo[h][0:64, :],
                                rr_sb[h][:],
                            )
                # stage + launch this head-pair's AllToAll chunk; chunk A
                # (hp=0) overlaps the hp=1 compute.
                a2a_in_v = a2a_in[hp][:].rearrange(
                    "(j g p) n -> j g p n", j=NCORES, g=2
                )
                for jj in range(NCORES):
                    sl = slice(NSLICE * (jj % 4), NSLICE * (jj % 4 + 1))
                    for g in range(2):
                        nc.sync.dma_start(
                            a2a_in_v[jj, g], xt_sb[2 * hp + g][:, sl]
                        )
                nc.gpsimd.collective_compute(
                    "AllToAll",
                    mybir.AluOpType.bypass,
                    replica_groups=GROUPS,
                    ins=[a2a_in[hp][:]],
                    outs=[a2a_out[hp][:]],
                )

            # ---- phase 3: output projection ----------------------------
            # contraction over 16 ct tiles: 8 from chunk A, then 8 from
            # chunk B (weight rows host-reordered/zeroed to match).
            CT_D = CH_ROWS // 128      # 8 per chunk
            pj = [ps.tile([128, DIM], F32, tag=PST[nt], name=f"pj{nt}")
                  for nt in range(NT)]
            wp_v = wp_e[:].rearrange("(ct p) c -> p ct c", p=128)
            for hp in range(2):
                a2a_out_v = a2a_out[hp][:].rearrange(
                    "(ct p) n -> p ct n", p=128
                )
                for ct in range(CT_D):
                    gct = hp * CT_D + ct
                    aa_t = p3pool.tile([128, NSLICE], BF16, tag="aa",
                                       name="aa")
                    nc.sync.dma_start(aa_t[:], a2a_out_v[:, ct, :])
                    wp_t = p3pool.tile([128, DIM], BF16, tag="wp", name="wp")
                    nc.sync.dma_start(wp_t[:], wp_v[:, gct, :])
                    for nt in range(NT):
                        for ch in range(2):
                            nc.tensor.matmul(
                                pj[nt][:, 512 * ch:512 * (ch + 1)],
                                aa_t[:, 128 * nt:128 * (nt + 1)],
                                wp_t[:, 512 * ch:512 * (ch + 1)],
                                start=(gct == 0), stop=(gct == 2 * CT_D - 1),
                            )
            for nt in range(NT):
                o_t = opool.tile([128, DIM], F32, tag="ot", name="ot")
                for ch in range(2):
                    csl = slice(512 * ch, 512 * (ch + 1))
                    nc.vector.tensor_add(
                        o_t[:, csl], pj[nt][:, csl], bpr_sb[:, csl]
                    )
                nc.sync.dma_start(out_e[128 * nt:128 * (nt + 1), :], o_t[:])

    if split_waits:
        _split_sync_waits(nc)
    return nc


def make_in_maps(q, k, v, mask, Wq, Wk, Wv, Wp, bp, N=N_FULL):
    """Shard + pre-transpose + bf16-cast the full inputs for the 8 cores."""
    bf = lambda a: np.ascontiguousarray(a).astype(BF16_NP)
    bp_rep = np.ascontiguousarray(
        np.broadcast_to(bp.astype(np.float32), (128, DIM))
    )
    in_maps = []
    for c in range(NCORES):
        b, r = divmod(c, 4)
        cs = slice(COLS * r, COLS * (r + 1))
        # wp rows in chunked-AllToAll channel order: [chunk, src rank j,
        # 128 ch]; src rank j delivers heads {4(j%4)+2c, +1} = channels
        # 256(j%4)+128c.. of ITS batch -- zero rows for other-quad sources.
        wp_re = np.zeros((2 * DIM, DIM), np.float32)
        for ck in range(2):
            for j in range(NCORES):
                if j // 4 == b:
                    src = 256 * (j % 4) + 128 * ck
                    dst = 128 * (NCORES * ck + j)
                    wp_re[dst:dst + 128] = Wp[src:src + 128]
        in_maps.append({
            "xqT": bf(q[b].T),
            "xkT": bf(k[b].T),
            "xvT": bf(v[b].T),
            "wq": bf(Wq[:, cs]),
            "wk": bf(Wk[:, cs]),
            "wv": bf(Wv[:, cs]),
            "wp": bf(wp_re),
            "maskT": bf((mask[b, 0].T.astype(np.float32) - 1.0) * 1e6),
            "bp_rep": bp_rep,
        })
    return in_maps


def assemble_out(results, N=N_FULL):
    NSLICE = N // 4
    out = np.empty((B, N, DIM), np.float32)
    for c in range(NCORES):
        b, r = divmod(c, 4)
        out[b, NSLICE * r:NSLICE * (r + 1), :] = results[c]["out"]
    return out


_NC_CACHE = {}


def _get_nc():
    if "nc" not in _NC_CACHE:
        _NC_CACHE["nc"] = build_nc()
    return _NC_CACHE["nc"]


def kernel(q, k, v, mask, Wq, Wk, Wv, Wp, bp):
    from concourse.bass_utils import run_bass_kernel_spmd

    q, k, v = (np.asarray(a, np.float32) for a in (q, k, v))
    mask = np.asarray(mask)
    Wq, Wk, Wv, Wp, bp = (
        np.asarray(a, np.float32) for a in (Wq, Wk, Wv, Wp, bp)
    )
    nc = _get_nc()
    in_maps = make_in_maps(q, k, v, mask, Wq, Wk, Wv, Wp, bp)
    res = run_bass_kernel_spmd(nc, in_maps, core_ids=list(range(NCORES)))
    return assemble_out(res.results)


# revision 29
# speedup vs baseline: 1.2694x; 1.1003x over previous
"""Multi-head attention (B=2, N=2048, DIM=1024, H=16) on 8 Trainium2 NeuronCores.

Sharding: tensor-parallel by head within two quads (cores 0-3 -> batch 0,
cores 4-7 -> batch 1; quad rank r owns heads 4r..4r+3). Each core computes
Q/K/V projections for its 4 heads, masked-softmax attention, then a
quad-local AllToAll re-shards the per-head attention output x^T from
head-split to sequence-split; each core runs the output projection (+bias)
for a disjoint 512-token slice and returns that output shard. The host only
shards inputs (transpose + bf16 cast) and concatenates the 8 output shards.

The head-pair loop (hp) is outermost so the AllToAll can be issued in two
chunks: chunk A (heads 0-1 of each rank) goes out while the PE computes
heads 2-3, hiding most of the collective latency. The output projection
accumulates chunk-A channels first, then chunk-B channels, with the weight
rows pre-reordered host-side to match the interleaved channel order.

Numerics: matmuls in bf16 with fp32 PSUM accumulation; softmax computed as
exp(SCALE*S)*mask / sum(exp(SCALE*S)*mask) without max-subtraction (scores
are ~N(0,1); exp never overflows fp32). Denominators come from an extra
ones-column appended to V in the attn@V matmul (row 64 of the 65-partition
VO accumulator). Normalization runs off-PSUM: reciprocal_approx_fast on the
denominator row, a bf16 ones-broadcast matmul, and one DVE multiply reading
the VO PSUM directly (no intermediate eviction).
Measured end-to-end L2 relative error vs the f32 reference: ~6e-3.
"""

import numpy as np
import ml_dtypes

import concourse.bass as bass
import concourse.mybir as mybir
import concourse.tile as tile

F32 = mybir.dt.float32
BF16 = mybir.dt.bfloat16
BF16_NP = ml_dtypes.bfloat16

B, DIM, H = 2, 1024, 16
N_FULL = 2048
HD = DIM // H          # 64
SCALE = HD ** -0.5     # 0.125
NCORES = 8
H_LOC = H // 4         # 4 heads per core
COLS = H_LOC * HD      # 256 local channels
KT_D = DIM // 128      # 8 contraction tiles over DIM
GROUPS = [list(range(NCORES))]   # AllToAll needs mesh routing (>4 ranks)
QUAD = 4


# ---------------------------------------------------------------------------
# Workaround: this walrus build rejects >2 sync waits on one instruction
# ("Too many sync wait commands" in setupSyncWait). The TileContext final
# drain aggregates one wait per logical processor; split it into a chain of
# single-wait drains.
# ---------------------------------------------------------------------------
def _patch_tile_drain():
    from bass_rust import ScopedClock

    if getattr(tile.TileContext, "_drain_patched", False):
        return

    def _drain_and_barrier(self, tick_clock, wait_clock):
        nc = self.nc
        drain_inst = nc.sync.drain()
        wait_clock.add_sem_waits(
            drain_inst.ins, ScopedClock({None: tick_clock.global_clock})
        )
        si = drain_inst.ins.sync_info
        if si is not None and len(si.on_wait) > 1:
            waits = list(si.on_wait)
            drain_inst.ins.sync_info = mybir.SyncInfo(
                on_wait=waits[:1], on_update=list(si.on_update)
            )
            for w in waits[1:]:
                d = nc.sync.drain()
                dsi = d.ins.sync_info
                upd = list(dsi.on_update) if dsi is not None else []
                d.ins.sync_info = mybir.SyncInfo(on_wait=[w], on_update=upd)

        nc.all_engine_barrier()
        assert self.sems is not None
        popped = nc._tile_sem_poison_stack.pop()
        assert popped is self._sem_poison
        nc.clear_and_free_semaphores(list(self.sems.allocated().values()))
        nc.all_engine_barrier()

    tile.TileContext._drain_and_barrier = _drain_and_barrier
    tile.TileContext._drain_patched = True


def _split_sync_waits(nc, maxw=1):
    """Walrus in this build rejects instructions carrying more than a couple
    of semaphore waits. Move excess waits onto injected same-engine NoOps
    immediately before the instruction (identical semantics: the engine
    blocks at the nop instead of at the instruction itself)."""
    n_split = 0
    for f in nc.m.functions:
        for bb in f.blocks:
            new_insts = []
            for ins in bb.instructions:
                si = ins.sync_info
                if si is not None and len(si.on_wait) > maxw:
                    waits = list(si.on_wait)
                    for i, w in enumerate(waits[maxw:]):
                        nop = mybir.InstNoOp(
                            name=f"{ins.name}-w{i}", ins=[], outs=[]
                        )
                        nop.engine = ins.engine
                        nop.sync_info = mybir.SyncInfo(
                            on_wait=[w], on_update=[]
                        )
                        new_insts.append(nop)
                    ins.sync_info = mybir.SyncInfo(
                        on_wait=waits[:maxw], on_update=list(si.on_update)
                    )
                    n_split += 1
                new_insts.append(ins)
            bb.instructions = new_insts
    return n_split


def build_nc(N=N_FULL, split_waits=True):
    """Build the per-core Bass program (same SPMD program for all 8 cores).

    N is parameterizable (multiple of 512) so a scaled-down variant can be
    validated in the simulator; the graded configuration is N=2048.
    """
    _patch_tile_drain()
    assert N % 512 == 0
    NSLICE = N // 4            # output rows per core
    MT = N // 128              # m-tiles over keys
    HS = min(N, 1024)          # attention n-chunk size
    NH = N // HS               # number of n-chunks in phase 2
    NT = NSLICE // 128         # output row tiles
    NCH = N // 512             # 512-col chunks of N
    HC = HS // 512             # 512-col chunks of one n-chunk

    nc = bass.Bass(trn_type="TRN2", num_devices=NCORES)

    xqT_e = nc.declare_dram_parameter("xqT", [DIM, N], BF16, isOutput=False)
    xkT_e = nc.declare_dram_parameter("xkT", [DIM, N], BF16, isOutput=False)
    xvT_e = nc.declare_dram_parameter("xvT", [DIM, N], BF16, isOutput=False)
    wq_e = nc.declare_dram_parameter("wq", [DIM, COLS], BF16, isOutput=False)
    wk_e = nc.declare_dram_parameter("wk", [DIM, COLS], BF16, isOutput=False)
    wv_e = nc.declare_dram_parameter("wv", [DIM, COLS], BF16, isOutput=False)
    # wp rows host-reordered/zero-padded to the chunked AllToAll channel
    # order: chunk A (heads 0-1 of each source rank) then chunk B (heads
    # 2-3), with other-quad source rows zeroed (their payload is the other
    # batch's data).
    wp_e = nc.declare_dram_parameter("wp", [2 * DIM, DIM], BF16, isOutput=False)
    maskT_e = nc.declare_dram_parameter("maskT", [N, N], BF16, isOutput=False)
    bpr_e = nc.declare_dram_parameter("bp_rep", [128, DIM], F32, isOutput=False)
    out_e = nc.declare_dram_parameter("out", [NSLICE, DIM], F32, isOutput=True)

    # per-chunk AllToAll buffers: 2 heads x 8 ranks, [8 dest * 2 g * 64, NSLICE]
    CH_ROWS = NCORES * 2 * HD    # 1024
    a2a_in = [nc.dram_tensor(f"a2a_in{c}", [CH_ROWS, NSLICE], BF16)
              for c in range(2)]
    a2a_out = [nc.dram_tensor(f"a2a_out{c}", [CH_ROWS, NSLICE], BF16)
               for c in range(2)]
    rscr = [nc.dram_tensor(f"rscr{h}", [1, min(N, 1024)], F32)
            for h in range(2)]
    rscr2 = [nc.dram_tensor(f"rscr2{h}", [1, min(N, 1024)], F32)
             for h in range(2)]

    with tile.TileContext(nc) as tc:
        with (
            tc.tile_pool(name="cpool", bufs=1) as cpool,
            tc.tile_pool(name="xstream", bufs=2) as xpool,
            tc.tile_pool(name="pupool", bufs=3) as pupool,
            tc.tile_pool(name="pupool2", bufs=2) as pupool2,
            tc.tile_pool(name="p3pool", bufs=3) as p3pool,
            tc.tile_pool(name="opool", bufs=2) as opool,
            tc.tile_pool(name="ps", bufs=1, space="PSUM") as ps,
        ):
            # PSUM: four 2-bank (4KB/partition) tag slots shared by all
            # phases; static pool allocation = 8 banks.
            PST = [f"PS{i}" for i in range(4)]

            # ---- long-lived SBUF tensors -------------------------------
            qt_sb = [cpool.tile([128, N], BF16, tag=f"qt{i}", name=f"qt{i}")
                     for i in range(2)]
            kt_sb = [cpool.tile([128, N], BF16, tag=f"kt{i}", name=f"kt{i}")
                     for i in range(2)]
            # V per m-tile: [m, head, 65]; cols 0..63 = V_head, col 64 = ones
            vt_sb = [cpool.tile([128, H_LOC, 65], BF16, tag=f"vt{t}",
                                name=f"vt{t}")
                     for t in range(MT)]
            # per-local-head attention output x^T, partitions 0..63
            xt_sb = [cpool.tile([64, N], BF16, tag=f"xth{g}", name=f"xth{g}")
                     for g in range(H_LOC)]
            dl_sb = [cpool.tile([1, HS], F32, tag=f"dl{h}", name=f"dl{h}")
                     for h in range(2)]
            r8_sb = [cpool.tile([128, 2 * (HS // 128)], F32, tag=f"r8{h}",
                                name=f"r8{h}")
                     for h in range(2)]
            rr_sb = [cpool.tile([64, HS], F32, tag=f"rr{h}", name=f"rr{h}")
                     for h in range(2)]
            mask_sb = cpool.tile([128, MT, N], BF16, tag="mask", name="mask")
            bpr_sb = cpool.tile([128, DIM], F32, tag="bpr", name="bpr")
            wq_sb = cpool.tile([128, KT_D, COLS], BF16, tag="wq", name="wq")
            wk_sb = cpool.tile([128, KT_D, COLS], BF16, tag="wk", name="wk")
            wv_sb = cpool.tile([128, KT_D, COLS], BF16, tag="wv", name="wv")
            xv_sb = cpool.tile([128, KT_D, N], BF16, tag="xv", name="xv")

            # weights + constants
            wq_v = wq_e[:].rearrange("(kt p) c -> p kt c", p=128)
            wk_v = wk_e[:].rearrange("(kt p) c -> p kt c", p=128)
            wv_v = wv_e[:].rearrange("(kt p) c -> p kt c", p=128)
            nc.sync.dma_start(wq_sb[:], wq_v)
            nc.sync.dma_start(wk_sb[:], wk_v)
            nc.sync.dma_start(wv_sb[:], wv_v)
            nc.sync.dma_start(bpr_sb[:], bpr_e[:])
            for t in range(MT):
                nc.gpsimd.memset(vt_sb[t][:, :, 64:65], 1.0)

            # ---- phase 1: projections ----------------------------------
            # Q^T and K^T: [COLS, N] as two 128-row blocks; kt-outer with
            # one live [128, <=1024] psum accumulator per (block, n-half).
            NH2 = max(1, N // 1024)
            W2 = min(N, 1024)
            for w_sb, x_e, dst in (
                (wq_sb, xqT_e, qt_sb),
                (wk_sb, xkT_e, kt_sb),
            ):
                psums = [ps.tile([128, W2], F32, tag=PST[cb * NH2 + n2],
                                 name="p1qk")
                         for cb in range(2) for n2 in range(NH2)]
                for kt in range(KT_D):
                    xt_t = xpool.tile([128, N], BF16, tag="xs", name="xs")
                    nc.sync.dma_start(xt_t[:], x_e[128 * kt:128 * (kt + 1), :])
                    for cb in range(2):
                        for nch in range(NCH):
                            n2, ch = divmod(nch, W2 // 512)
                            nc.tensor.matmul(
                                psums[cb * NH2 + n2][:, 512 * ch:512 * (ch + 1)],
                                w_sb[:, kt, 128 * cb:128 * (cb + 1)],
                                xt_t[:, 512 * nch:512 * (nch + 1)],
                                start=(kt == 0), stop=(kt == KT_D - 1),
                            )
                for cb in range(2):
                    for n2 in range(NH2):
                        nc.scalar.copy(
                            dst[cb][:, W2 * n2:W2 * (n2 + 1)],
                            psums[cb * NH2 + n2][:],
                        )

            # V in natural layout: out[m-tile, 4*HD] = xvT_kt^T @ wv_kt
            xv_v = xvT_e[:].rearrange("(kt p) n -> p kt n", p=128)
            nc.sync.dma_start(xv_sb[:], xv_v)
            for t in range(MT):
                nc.sync.dma_start(
                    mask_sb[:, t, :], maskT_e[128 * t:128 * (t + 1), :]
                )
            for t in range(MT):
                vps = ps.tile([128, COLS], F32, tag=PST[t % 2], name="p1v")
                for kt in range(KT_D):
                    nc.tensor.matmul(
                        vps[:],
                        xv_sb[:, kt, 128 * t:128 * (t + 1)],
                        wv_sb[:, kt, :],
                        start=(kt == 0), stop=(kt == KT_D - 1),
                    )
                nc.scalar.copy(
                    vt_sb[t][:, :, 0:HD],
                    vps[:].rearrange("p (h d) -> p h d", h=H_LOC),
                )

            # ---- phase 2: attention (hp outer for chunked collective) --
            # Scores in PSUM; exp on ScalarE; multiplicative mask on DVE
            # (bf16 all-SBUF, removes the identity-matmul mask add from the
            # PE); attn@[V|ones] accumulation issued one t-iteration late so
            # the PE never stalls on the exp->mask chain; normalization
            # reads the VO accumulator straight from PSUM.
            KPL = HS // 128            # reciprocal reshape columns

            def emit_scores(hp, nh, t):
                s_ps = [ps.tile([128, HS], F32, tag=PST[h], name="s")
                        for h in range(2)]
                # score matmul pairs adjacent so the K=64 row-group
                # concurrency engages (measured 1.75x vs serial)
                for ch in range(HC):
                    csl = slice(512 * ch, 512 * (ch + 1))
                    gsl = slice(HS * nh + 512 * ch, HS * nh + 512 * (ch + 1))
                    for h in range(2):
                        nc.tensor.matmul(
                            s_ps[h][:, csl],
                            kt_sb[hp][64 * h:64 * (h + 1),
                                      128 * t:128 * (t + 1)],
                            qt_sb[hp][64 * h:64 * (h + 1), gsl],
                            start=True, stop=True,
                            tile_position=(64 * h, 0),
                        )
                pu2s = []
                for h in range(2):
                    pu = pupool.tile([128, HS], BF16, tag="pu", name="pu")
                    nc.scalar.activation(
                        pu[:], s_ps[h][:],
                        mybir.ActivationFunctionType.Exp,
                        scale=float(SCALE),
                    )
                    pu2 = pupool2.tile([128, HS], BF16, tag="pu2", name="pu2")
                    gslf = slice(HS * nh, HS * (nh + 1))
                    nc.vector.tensor_mul(
                        pu2[:], pu[:], mask_sb[:, t, gslf]
                    )
                    pu2s.append(pu2)
                return pu2s

            def emit_av(hp, nh, t, vo, pu2s):
                for h in range(2):
                    for ch in range(HC):
                        csl = slice(512 * ch, 512 * (ch + 1))
                        nc.tensor.matmul(
                            vo[h][:, csl],
                            vt_sb[t][:, 2 * hp + h, :],
                            pu2s[h][:, csl],
                            start=(t == 0), stop=(t == MT - 1),
                        )

            for hp in range(2):
                for nh in range(NH):
                    nsl = slice(HS * nh, HS * (nh + 1))
                    vo = [ps.tile([65, HS], F32, tag=PST[2 + h], name="vo")
                          for h in range(2)]
                    prev = emit_scores(hp, nh, 0)
                    for t in range(1, MT):
                        cur = emit_scores(hp, nh, t)
                        emit_av(hp, nh, t - 1, vo, prev)
                        prev = cur
                    emit_av(hp, nh, MT - 1, vo, prev)
                    # normalize: denominator row -> DRAM -> [128, KPL]
                    # reshape -> 128-lane reciprocal -> DRAM -> broadcast
                    # DMA to 64 partitions -> one DVE multiply (vo stays in
                    # PSUM; score PSUM tags are never touched).
                    for h in range(2):
                        nc.vector.tensor_copy(dl_sb[h][:], vo[h][64:65, :])
                        nc.sync.dma_start(rscr[h][:], dl_sb[h][:])
                        rv = rscr[h][:].rearrange("o (p k) -> (o p) k", p=128)
                        nc.sync.dma_start(r8_sb[h][:, 0:KPL], rv)
                        nc.vector.reciprocal(
                            r8_sb[h][:, KPL:2 * KPL], r8_sb[h][:, 0:KPL]
                        )
                        rv2 = rscr2[h][:].rearrange(
                            "o (p k) -> (o p) k", p=128
                        )
                        nc.sync.dma_start(rv2, r8_sb[h][:, KPL:2 * KPL])
                        nc.sync.dma_start(
                            rr_sb[h][:], rscr2[h][:].broadcast_to((64, HS))
                        )
                        with nc.allow_low_precision(reason="softmax y bf16"):
                            nc.vector.tensor_mul(
                                xt_sb[2 * hp + h][:, nsl],
                                vo[h][0:64, :],
                                rr_sb[h][:],
                            )
                # stage + launch this head-pair's AllToAll chunk; chunk A
                # (hp=0) overlaps the hp=1 compute.
                a2a_in_v = a2a_in[hp][:].rearrange(
                    "(j g p) n -> j g p n", j=NCORES, g=2
                )
                for jj in range(NCORES):
                    sl = slice(NSLICE * (jj % 4), NSLICE * (jj % 4 + 1))
                    for g in range(2):
                        nc.sync.dma_start(
                            a2a_in_v[jj, g], xt_sb[2 * hp + g][:, sl]
                        )
                nc.gpsimd.collective_compute(
                    "AllToAll",
                    mybir.AluOpType.bypass,
                    replica_groups=GROUPS,
                    ins=[a2a_in[hp][:]],
                    outs=[a2a_out[hp][:]],
                )

            # ---- phase 3: output projection ----------------------------
            # contraction over 16 ct tiles: 8 from chunk A, then 8 from
            # chunk B (weight rows host-reordered/zeroed to match).
            CT_D = CH_ROWS // 128      # 8 per chunk
            pj = [ps.tile([128, DIM], F32, tag=PST[nt], name=f"pj{nt}")
                  for nt in range(NT)]
            wp_v = wp_e[:].rearrange("(ct p) c -> p ct c", p=128)
            for hp in range(2):
                a2a_out_v = a2a_out[hp][:].rearrange(
                    "(ct p) n -> p ct n", p=128
                )
                for ct in range(CT_D):
                    gct = hp * CT_D + ct
                    aa_t = p3pool.tile([128, NSLICE], BF16, tag="aa",
                                       name="aa")
                    nc.sync.dma_start(aa_t[:], a2a_out_v[:, ct, :])
                    wp_t = p3pool.tile([128, DIM], BF16, tag="wp", name="wp")
                    nc.sync.dma_start(wp_t[:], wp_v[:, gct, :])
                    for nt in range(NT):
                        for ch in range(2):
                            nc.tensor.matmul(
                                pj[nt][:, 512 * ch:512 * (ch + 1)],
                                aa_t[:, 128 * nt:128 * (nt + 1)],
                                wp_t[:, 512 * ch:512 * (ch + 1)],
                                start=(gct == 0), stop=(gct == 2 * CT_D - 1),
                            )
            for nt in range(NT):
                o_t = opool.tile([128, DIM], F32, tag="ot", name="ot")
                for ch in range(2):
                    csl = slice(512 * ch, 512 * (ch + 1))
                    nc.vector.tensor_add(
                        o_t[:, csl], pj[nt][:, csl], bpr_sb[:, csl]
                    )
                nc.sync.dma_start(out_e[128 * nt:128 * (nt + 1), :], o_t[:])

    if split_waits:
        _split_sync_waits(nc)
    return nc


def make_in_maps(q, k, v, mask, Wq, Wk, Wv, Wp, bp, N=N_FULL):
    """Shard + pre-transpose + bf16-cast the full inputs for the 8 cores."""
    bf = lambda a: np.ascontiguousarray(a).astype(BF16_NP)
    bp_rep = np.ascontiguousarray(
        np.broadcast_to(bp.astype(np.float32), (128, DIM))
    )
    in_maps = []
    for c in range(NCORES):
        b, r = divmod(c, 4)
        cs = slice(COLS * r, COLS * (r + 1))
        # wp rows in chunked-AllToAll channel order: [chunk, src rank j,
        # 128 ch]; src rank j delivers heads {4(j%4)+2c, +1} = channels
        # 256(j%4)+128c.. of ITS batch -- zero rows for other-quad sources.
        wp_re = np.zeros((2 * DIM, DIM), np.float32)
        for ck in range(2):
            for j in range(NCORES):
                if j // 4 == b:
                    src = 256 * (j % 4) + 128 * ck
                    dst = 128 * (NCORES * ck + j)
                    wp_re[dst:dst + 128] = Wp[src:src + 128]
        in_maps.append({
            "xqT": bf(q[b].T),
            "xkT": bf(k[b].T),
            "xvT": bf(v[b].T),
            "wq": bf(Wq[:, cs]),
            "wk": bf(Wk[:, cs]),
            "wv": bf(Wv[:, cs]),
            "wp": bf(wp_re),
            "maskT": bf(mask[b, 0].T.astype(np.float32)),
            "bp_rep": bp_rep,
        })
    return in_maps


def assemble_out(results, N=N_FULL):
    NSLICE = N // 4
    out = np.empty((B, N, DIM), np.float32)
    for c in range(NCORES):
        b, r = divmod(c, 4)
        out[b, NSLICE * r:NSLICE * (r + 1), :] = results[c]["out"]
    return out


_NC_CACHE = {}


def _get_nc():
    if "nc" not in _NC_CACHE:
        _NC_CACHE["nc"] = build_nc()
    return _NC_CACHE["nc"]


def kernel(q, k, v, mask, Wq, Wk, Wv, Wp, bp):
    from concourse.bass_utils import run_bass_kernel_spmd

    q, k, v = (np.asarray(a, np.float32) for a in (q, k, v))
    mask = np.asarray(mask)
    Wq, Wk, Wv, Wp, bp = (
        np.asarray(a, np.float32) for a in (Wq, Wk, Wv, Wp, bp)
    )
    nc = _get_nc()
    in_maps = make_in_maps(q, k, v, mask, Wq, Wk, Wv, Wp, bp)
    res = run_bass_kernel_spmd(nc, in_maps, core_ids=list(range(NCORES)))
    return assemble_out(res.results)


# revision 33
# speedup vs baseline: 1.3464x; 1.0607x over previous
"""Multi-head attention (B=2, N=2048, DIM=1024, H=16) on 8 Trainium2 NeuronCores.

Sharding: tensor-parallel by head within two quads (cores 0-3 -> batch 0,
cores 4-7 -> batch 1; quad rank r owns heads 4r..4r+3). Each core computes
Q/K/V projections for its 4 heads, masked-softmax attention, then a
quad-local AllToAll re-shards the per-head attention output x^T from
head-split to sequence-split; each core runs the output projection (+bias)
for a disjoint 512-token slice and returns that output shard. The host only
shards inputs (transpose + bf16 cast) and concatenates the 8 output shards.

The head-pair loop (hp) is outermost so the AllToAll can be issued in two
chunks: chunk A (heads 0-1 of each rank) goes out while the PE computes
heads 2-3, hiding most of the collective latency. The output projection
accumulates chunk-A channels first, then chunk-B channels, with the weight
rows pre-reordered host-side to match the interleaved channel order.

Numerics: matmuls in bf16 with fp32 PSUM accumulation; softmax computed as
exp(SCALE*S)*mask / sum(exp(SCALE*S)*mask) without max-subtraction (scores
are ~N(0,1); exp never overflows fp32). Denominators come from an extra
ones-column appended to V in the attn@V matmul (row 64 of the 65-partition
VO accumulator). Normalization runs off-PSUM: reciprocal_approx_fast on the
denominator row, a bf16 ones-broadcast matmul, and one DVE multiply reading
the VO PSUM directly (no intermediate eviction).
Measured end-to-end L2 relative error vs the f32 reference: ~6e-3.
"""

import numpy as np
import ml_dtypes

import concourse.bass as bass
import concourse.mybir as mybir
import concourse.tile as tile

F32 = mybir.dt.float32
BF16 = mybir.dt.bfloat16
BF16_NP = ml_dtypes.bfloat16

B, DIM, H = 2, 1024, 16
N_FULL = 2048
HD = DIM // H          # 64
SCALE = HD ** -0.5     # 0.125
NCORES = 8
H_LOC = H // 4         # 4 heads per core
COLS = H_LOC * HD      # 256 local channels
KT_D = DIM // 128      # 8 contraction tiles over DIM
GROUPS = [list(range(NCORES))]   # AllToAll needs mesh routing (>4 ranks)
QUAD = 4


# ---------------------------------------------------------------------------
# Workaround: this walrus build rejects >2 sync waits on one instruction
# ("Too many sync wait commands" in setupSyncWait). The TileContext final
# drain aggregates one wait per logical processor; split it into a chain of
# single-wait drains.
# ---------------------------------------------------------------------------
def _patch_tile_drain():
    from bass_rust import ScopedClock

    if getattr(tile.TileContext, "_drain_patched", False):
        return

    def _drain_and_barrier(self, tick_clock, wait_clock):
        nc = self.nc
        drain_inst = nc.sync.drain()
        wait_clock.add_sem_waits(
            drain_inst.ins, ScopedClock({None: tick_clock.global_clock})
        )
        si = drain_inst.ins.sync_info
        if si is not None and len(si.on_wait) > 1:
            waits = list(si.on_wait)
            drain_inst.ins.sync_info = mybir.SyncInfo(
                on_wait=waits[:1], on_update=list(si.on_update)
            )
            for w in waits[1:]:
                d = nc.sync.drain()
                dsi = d.ins.sync_info
                upd = list(dsi.on_update) if dsi is not None else []
                d.ins.sync_info = mybir.SyncInfo(on_wait=[w], on_update=upd)

        nc.all_engine_barrier()
        assert self.sems is not None
        popped = nc._tile_sem_poison_stack.pop()
        assert popped is self._sem_poison
        nc.clear_and_free_semaphores(list(self.sems.allocated().values()))
        nc.all_engine_barrier()

    tile.TileContext._drain_and_barrier = _drain_and_barrier
    tile.TileContext._drain_patched = True


def _split_sync_waits(nc, maxw=1):
    """Walrus in this build rejects instructions carrying more than a couple
    of semaphore waits. Move excess waits onto injected same-engine NoOps
    immediately before the instruction (identical semantics: the engine
    blocks at the nop instead of at the instruction itself)."""
    n_split = 0
    for f in nc.m.functions:
        for bb in f.blocks:
            new_insts = []
            for ins in bb.instructions:
                si = ins.sync_info
                if si is not None and len(si.on_wait) > maxw:
                    waits = list(si.on_wait)
                    for i, w in enumerate(waits[maxw:]):
                        nop = mybir.InstNoOp(
                            name=f"{ins.name}-w{i}", ins=[], outs=[]
                        )
                        nop.engine = ins.engine
                        nop.sync_info = mybir.SyncInfo(
                            on_wait=[w], on_update=[]
                        )
                        new_insts.append(nop)
                    ins.sync_info = mybir.SyncInfo(
                        on_wait=waits[:maxw], on_update=list(si.on_update)
                    )
                    n_split += 1
                new_insts.append(ins)
            bb.instructions = new_insts
    return n_split


def build_nc(N=N_FULL, split_waits=True):
    """Build the per-core Bass program (same SPMD program for all 8 cores).

    N is parameterizable (multiple of 512) so a scaled-down variant can be
    validated in the simulator; the graded configuration is N=2048.
    """
    _patch_tile_drain()
    assert N % 512 == 0
    NSLICE = N // 4            # output rows per core
    MT = N // 128              # m-tiles over keys
    HS = min(N, 1024)          # attention n-chunk size
    NH = N // HS               # number of n-chunks in phase 2
    NT = NSLICE // 128         # output row tiles
    NCH = N // 512             # 512-col chunks of N
    HC = HS // 512             # 512-col chunks of one n-chunk

    nc = bass.Bass(trn_type="TRN2", num_devices=NCORES)

    xqT_e = nc.declare_dram_parameter("xqT", [DIM, N], BF16, isOutput=False)
    xkT_e = nc.declare_dram_parameter("xkT", [DIM, N], BF16, isOutput=False)
    xvT_e = nc.declare_dram_parameter("xvT", [DIM, N], BF16, isOutput=False)
    wq_e = nc.declare_dram_parameter("wq", [DIM, COLS], BF16, isOutput=False)
    wk_e = nc.declare_dram_parameter("wk", [DIM, COLS], BF16, isOutput=False)
    wv_e = nc.declare_dram_parameter("wv", [DIM, COLS], BF16, isOutput=False)
    # wp rows host-reordered/zero-padded to the chunked AllToAll channel
    # order: chunk A (heads 0-1 of each source rank) then chunk B (heads
    # 2-3), with other-quad source rows zeroed (their payload is the other
    # batch's data).
    wp_e = nc.declare_dram_parameter("wp", [2 * DIM, DIM], BF16, isOutput=False)
    maskT_e = nc.declare_dram_parameter("maskT", [N, N], BF16, isOutput=False)
    bpr_e = nc.declare_dram_parameter("bp_rep", [128, DIM], F32, isOutput=False)
    out_e = nc.declare_dram_parameter("out", [NSLICE, DIM], F32, isOutput=True)

    # per-chunk AllToAll buffers: 2 heads x 8 ranks, [8 dest * 2 g * 64, NSLICE]
    CH_ROWS = NCORES * 2 * HD    # 1024
    a2a_in = [nc.dram_tensor(f"a2a_in{c}", [CH_ROWS, NSLICE], BF16)
              for c in range(2)]
    a2a_out = [nc.dram_tensor(f"a2a_out{c}", [CH_ROWS, NSLICE], BF16)
               for c in range(2)]
    rscr = [nc.dram_tensor(f"rscr{h}", [1, min(N, 1024)], BF16)
            for h in range(2)]
    rscr2 = [nc.dram_tensor(f"rscr2{h}", [1, min(N, 1024)], F32)
             for h in range(2)]

    with tile.TileContext(nc) as tc:
        with (
            tc.tile_pool(name="cpool", bufs=1) as cpool,
            tc.tile_pool(name="xstream", bufs=2) as xpool,
            tc.tile_pool(name="pupool", bufs=3) as pupool,
            tc.tile_pool(name="pupool2", bufs=2) as pupool2,
            tc.tile_pool(name="yupool", bufs=2) as yupool,
            tc.tile_pool(name="p3pool", bufs=3) as p3pool,
            tc.tile_pool(name="opool", bufs=2) as opool,
            tc.tile_pool(name="ps", bufs=1, space="PSUM") as ps,
        ):
            # PSUM: four 2-bank (4KB/partition) tag slots shared by all
            # phases; static pool allocation = 8 banks.
            PST = [f"PS{i}" for i in range(4)]

            # ---- long-lived SBUF tensors -------------------------------
            qt_sb = [cpool.tile([128, N], BF16, tag=f"qt{i}", name=f"qt{i}")
                     for i in range(2)]
            kt_sb = [cpool.tile([128, N], BF16, tag=f"kt{i}", name=f"kt{i}")
                     for i in range(2)]
            # V per m-tile: [m, head, 65]; cols 0..63 = V_head, col 64 = ones
            vt_sb = [cpool.tile([128, H_LOC, 65], BF16, tag=f"vt{t}",
                                name=f"vt{t}")
                     for t in range(MT)]
            # per-local-head attention output x^T, partitions 0..63
            xt_sb = [cpool.tile([64, N], BF16, tag=f"xth{g}", name=f"xth{g}")
                     for g in range(H_LOC)]
            r8b_sb = [cpool.tile([128, HS // 128], BF16, tag=f"r8b{h}",
                                 name=f"r8b{h}")
                      for h in range(2)]
            r8_sb = [cpool.tile([128, HS // 128], F32, tag=f"r8{h}",
                                name=f"r8{h}")
                     for h in range(2)]
            rr_sb = [cpool.tile([64, HS], F32, tag=f"rr{h}", name=f"rr{h}")
                     for h in range(2)]
            mask_sb = cpool.tile([128, MT, N], BF16, tag="mask", name="mask")
            bpr_sb = cpool.tile([128, DIM], F32, tag="bpr", name="bpr")
            wq_sb = cpool.tile([128, KT_D, COLS], BF16, tag="wq", name="wq")
            wk_sb = cpool.tile([128, KT_D, COLS], BF16, tag="wk", name="wk")
            wv_sb = cpool.tile([128, KT_D, COLS], BF16, tag="wv", name="wv")
            xv_sb = cpool.tile([128, KT_D, N], BF16, tag="xv", name="xv")

            # weights + constants
            wq_v = wq_e[:].rearrange("(kt p) c -> p kt c", p=128)
            wk_v = wk_e[:].rearrange("(kt p) c -> p kt c", p=128)
            wv_v = wv_e[:].rearrange("(kt p) c -> p kt c", p=128)
            nc.sync.dma_start(wq_sb[:], wq_v)
            nc.sync.dma_start(wk_sb[:], wk_v)
            nc.sync.dma_start(wv_sb[:], wv_v)
            nc.sync.dma_start(bpr_sb[:], bpr_e[:])
            for t in range(MT):
                nc.gpsimd.memset(vt_sb[t][:, :, 64:65], 1.0)

            # ---- phase 1: projections ----------------------------------
            # Q^T and K^T: [COLS, N] as two 128-row blocks; kt-outer with
            # one live [128, <=1024] psum accumulator per (block, n-half).
            NH2 = max(1, N // 1024)
            W2 = min(N, 1024)
            for w_sb, x_e, dst in (
                (wq_sb, xqT_e, qt_sb),
                (wk_sb, xkT_e, kt_sb),
            ):
                psums = [ps.tile([128, W2], F32, tag=PST[cb * NH2 + n2],
                                 name="p1qk")
                         for cb in range(2) for n2 in range(NH2)]
                for kt in range(KT_D):
                    xt_t = xpool.tile([128, N], BF16, tag="xs", name="xs")
                    nc.sync.dma_start(xt_t[:], x_e[128 * kt:128 * (kt + 1), :])
                    for cb in range(2):
                        for nch in range(NCH):
                            n2, ch = divmod(nch, W2 // 512)
                            nc.tensor.matmul(
                                psums[cb * NH2 + n2][:, 512 * ch:512 * (ch + 1)],
                                w_sb[:, kt, 128 * cb:128 * (cb + 1)],
                                xt_t[:, 512 * nch:512 * (nch + 1)],
                                start=(kt == 0), stop=(kt == KT_D - 1),
                            )
                for cb in range(2):
                    for n2 in range(NH2):
                        nc.scalar.copy(
                            dst[cb][:, W2 * n2:W2 * (n2 + 1)],
                            psums[cb * NH2 + n2][:],
                        )

            # V in natural layout: out[m-tile, 4*HD] = xvT_kt^T @ wv_kt
            xv_v = xvT_e[:].rearrange("(kt p) n -> p kt n", p=128)
            nc.sync.dma_start(xv_sb[:], xv_v)
            for t in range(MT):
                nc.sync.dma_start(
                    mask_sb[:, t, :], maskT_e[128 * t:128 * (t + 1), :]
                )
            for t in range(MT):
                vps = ps.tile([128, COLS], F32, tag=PST[t % 2], name="p1v")
                for kt in range(KT_D):
                    nc.tensor.matmul(
                        vps[:],
                        xv_sb[:, kt, 128 * t:128 * (t + 1)],
                        wv_sb[:, kt, :],
                        start=(kt == 0), stop=(kt == KT_D - 1),
                    )
                nc.scalar.copy(
                    vt_sb[t][:, :, 0:HD],
                    vps[:].rearrange("p (h d) -> p h d", h=H_LOC),
                )

            # ---- phase 2: attention (hp outer for chunked collective) --
            # Scores in PSUM; exp on ScalarE; multiplicative mask on DVE
            # (bf16 all-SBUF, removes the identity-matmul mask add from the
            # PE); attn@[V|ones] accumulation issued one t-iteration late so
            # the PE never stalls on the exp->mask chain; normalization
            # reads the VO accumulator straight from PSUM.
            KPL = HS // 128            # reciprocal reshape columns

            def emit_scores(hp, nh, t):
                s_ps = [ps.tile([128, HS], F32, tag=PST[h], name="s")
                        for h in range(2)]
                # score matmul pairs adjacent so the K=64 row-group
                # concurrency engages (measured 1.75x vs serial)
                for ch in range(HC):
                    csl = slice(512 * ch, 512 * (ch + 1))
                    gsl = slice(HS * nh + 512 * ch, HS * nh + 512 * (ch + 1))
                    for h in range(2):
                        nc.tensor.matmul(
                            s_ps[h][:, csl],
                            kt_sb[hp][64 * h:64 * (h + 1),
                                      128 * t:128 * (t + 1)],
                            qt_sb[hp][64 * h:64 * (h + 1), gsl],
                            start=True, stop=True,
                            tile_position=(64 * h, 0),
                        )
                pu2s = []
                for h in range(2):
                    pu = pupool.tile([128, HS], BF16, tag="pu", name="pu")
                    nc.scalar.activation(
                        pu[:], s_ps[h][:],
                        mybir.ActivationFunctionType.Exp,
                        scale=float(SCALE),
                    )
                    pu2 = pupool2.tile([128, HS], BF16, tag="pu2", name="pu2")
                    gslf = slice(HS * nh, HS * (nh + 1))
                    nc.vector.tensor_mul(
                        pu2[:], pu[:], mask_sb[:, t, gslf]
                    )
                    pu2s.append(pu2)
                return pu2s

            def emit_av(hp, nh, t, vo, pu2s):
                for h in range(2):
                    for ch in range(HC):
                        csl = slice(512 * ch, 512 * (ch + 1))
                        nc.tensor.matmul(
                            vo[h][:, csl],
                            vt_sb[t][:, 2 * hp + h, :],
                            pu2s[h][:, csl],
                            start=(t == 0), stop=(t == MT - 1),
                        )

            for hp in range(2):
                for nh in range(NH):
                    nsl = slice(HS * nh, HS * (nh + 1))
                    vo = [ps.tile([65, HS], F32, tag=PST[2 + h], name="vo")
                          for h in range(2)]
                    prev = emit_scores(hp, nh, 0)
                    for t in range(1, MT):
                        cur = emit_scores(hp, nh, t)
                        emit_av(hp, nh, t - 1, vo, prev)
                        prev = cur
                    emit_av(hp, nh, MT - 1, vo, prev)
                    # Evict VO to SBUF immediately (frees the PSUM tags so
                    # the next pass's AV never stalls), then normalize from
                    # SBUF entirely in the shadow: denominator row -> DRAM
                    # -> [128, KPL] reshape -> 128-lane reciprocal -> DRAM
                    # -> broadcast DMA to 64 partitions -> one DVE multiply.
                    yus = []
                    for h in range(2):
                        yu = yupool.tile([65, HS], BF16, tag="yu", name="yu")
                        with nc.allow_low_precision(reason="softmax y bf16"):
                            nc.scalar.copy(yu[:], vo[h][:])
                        yus.append(yu)
                    for h in range(2):
                        yu = yus[h]
                        nc.sync.dma_start(rscr[h][:], yu[64:65, :])
                        rv = rscr[h][:].rearrange("o (p k) -> (o p) k", p=128)
                        nc.sync.dma_start(r8b_sb[h][:], rv)
                        nc.vector.reciprocal(r8_sb[h][:], r8b_sb[h][:])
                        rv2 = rscr2[h][:].rearrange(
                            "o (p k) -> (o p) k", p=128
                        )
                        nc.sync.dma_start(rv2, r8_sb[h][:])
                        nc.sync.dma_start(
                            rr_sb[h][:], rscr2[h][:].broadcast_to((64, HS))
                        )
                        with nc.allow_low_precision(reason="softmax y bf16"):
                            nc.vector.tensor_mul(
                                xt_sb[2 * hp + h][:, nsl],
                                yu[0:64, :],
                                rr_sb[h][:],
                            )
                # stage + launch this head-pair's AllToAll chunk; chunk A
                # (hp=0) overlaps the hp=1 compute.
                a2a_in_v = a2a_in[hp][:].rearrange(
                    "(j g p) n -> j g p n", j=NCORES, g=2
                )
                for jj in range(NCORES):
                    sl = slice(NSLICE * (jj % 4), NSLICE * (jj % 4 + 1))
                    for g in range(2):
                        nc.sync.dma_start(
                            a2a_in_v[jj, g], xt_sb[2 * hp + g][:, sl]
                        )
                nc.gpsimd.collective_compute(
                    "AllToAll",
                    mybir.AluOpType.bypass,
                    replica_groups=GROUPS,
                    ins=[a2a_in[hp][:]],
                    outs=[a2a_out[hp][:]],
                )

            # ---- phase 3: output projection ----------------------------
            # contraction over 16 ct tiles: 8 from chunk A, then 8 from
            # chunk B (weight rows host-reordered/zeroed to match).
            CT_D = CH_ROWS // 128      # 8 per chunk
            pj = [ps.tile([128, DIM], F32, tag=PST[nt], name=f"pj{nt}")
                  for nt in range(NT)]
            wp_v = wp_e[:].rearrange("(ct p) c -> p ct c", p=128)
            for hp in range(2):
                a2a_out_v = a2a_out[hp][:].rearrange(
                    "(ct p) n -> p ct n", p=128
                )
                for ct in range(CT_D):
                    gct = hp * CT_D + ct
                    aa_t = p3pool.tile([128, NSLICE], BF16, tag="aa",
                                       name="aa")
                    nc.sync.dma_start(aa_t[:], a2a_out_v[:, ct, :])
                    wp_t = p3pool.tile([128, DIM], BF16, tag="wp", name="wp")
                    nc.sync.dma_start(wp_t[:], wp_v[:, gct, :])
                    for nt in range(NT):
                        for ch in range(2):
                            nc.tensor.matmul(
                                pj[nt][:, 512 * ch:512 * (ch + 1)],
                                aa_t[:, 128 * nt:128 * (nt + 1)],
                                wp_t[:, 512 * ch:512 * (ch + 1)],
                                start=(gct == 0), stop=(gct == 2 * CT_D - 1),
                            )
            for nt in range(NT):
                o_t = opool.tile([128, DIM], F32, tag="ot", name="ot")
                for ch in range(2):
                    csl = slice(512 * ch, 512 * (ch + 1))
                    nc.vector.tensor_add(
                        o_t[:, csl], pj[nt][:, csl], bpr_sb[:, csl]
                    )
                nc.sync.dma_start(out_e[128 * nt:128 * (nt + 1), :], o_t[:])

    if split_waits:
        _split_sync_waits(nc)
    return nc


def make_in_maps(q, k, v, mask, Wq, Wk, Wv, Wp, bp, N=N_FULL):
    """Shard + pre-transpose + bf16-cast the full inputs for the 8 cores."""
    bf = lambda a: np.ascontiguousarray(a).astype(BF16_NP)
    bp_rep = np.ascontiguousarray(
        np.broadcast_to(bp.astype(np.float32), (128, DIM))
    )
    in_maps = []
    for c in range(NCORES):
        b, r = divmod(c, 4)
        cs = slice(COLS * r, COLS * (r + 1))
        # wp rows in chunked-AllToAll channel order: [chunk, src rank j,
        # 128 ch]; src rank j delivers heads {4(j%4)+2c, +1} = channels
        # 256(j%4)+128c.. of ITS batch -- zero rows for other-quad sources.
        wp_re = np.zeros((2 * DIM, DIM), np.float32)
        for ck in range(2):
            for j in range(NCORES):
                if j // 4 == b:
                    src = 256 * (j % 4) + 128 * ck
                    dst = 128 * (NCORES * ck + j)
                    wp_re[dst:dst + 128] = Wp[src:src + 128]
        in_maps.append({
            "xqT": bf(q[b].T),
            "xkT": bf(k[b].T),
            "xvT": bf(v[b].T),
            "wq": bf(Wq[:, cs]),
            "wk": bf(Wk[:, cs]),
            "wv": bf(Wv[:, cs]),
            "wp": bf(wp_re),
            "maskT": bf(mask[b, 0].T.astype(np.float32)),
            "bp_rep": bp_rep,
        })
    return in_maps


def assemble_out(results, N=N_FULL):
    NSLICE = N // 4
    out = np.empty((B, N, DIM), np.float32)
    for c in range(NCORES):
        b, r = divmod(c, 4)
        out[b, NSLICE * r:NSLICE * (r + 1), :] = results[c]["out"]
    return out


_NC_CACHE = {}


def _get_nc():
    if "nc" not in _NC_CACHE:
        _NC_CACHE["nc"] = build_nc()
    return _NC_CACHE["nc"]


def kernel(q, k, v, mask, Wq, Wk, Wv, Wp, bp):
    from concourse.bass_utils import run_bass_kernel_spmd

    q, k, v = (np.asarray(a, np.float32) for a in (q, k, v))
    mask = np.asarray(mask)
    Wq, Wk, Wv, Wp, bp = (
        np.asarray(a, np.float32) for a in (Wq, Wk, Wv, Wp, bp)
    )
    nc = _get_nc()
    in_maps = make_in_maps(q, k, v, mask, Wq, Wk, Wv, Wp, bp)
    res = run_bass_kernel_spmd(nc, in_maps, core_ids=list(range(NCORES)))
    return assemble_out(res.results)
